# revision 30
# baseline (speedup 1.0000x reference)
"""CompressedLinear Trainium2 kernel.

Computes y = x @ (w_int8 * 0.01)^T + bias for
  x      [4, 32, 4096]  fp32
  w_int8 [11008, 4096]  int32 (int8 values)
  bias   [11008]        fp32
  y      [4, 32, 11008] fp32

Strategy (tensor-parallel over output rows, 8 NeuronCores):
- The weight payload is int8; stream it from HBM as 1 byte/element
  (5.6 MB/core instead of the baseline's 22.5 MB int32; measured DMA
  rate ~275 GB/s/core) and widen to bf16 on-chip. A single bf16 matmul
  pass suffices: int8 weights are exact in bf16 and x's bf16 rounding
  gives rel err ~1.5e-3 (tolerance is 2e-2); the baseline's hi/lo
  double pass is unnecessary.
- The int8->bf16 widening is split across the two fast elementwise
  engines (measured: DVE 0.74 ns/col, ACT 1.08 ns/col; gpsimd is 5x
  slower and the SDMA cast path costs 2.4x raw streaming, so neither
  is used): rows [0,D) ride the sync HWDGE queue and convert on DVE,
  rows [D,RPC) ride the scalar HWDGE queue and convert on ACT. Row
  groups are separate host-prepared tensors so every DMA is a flat,
  fully contiguous identity copy (2-8 chunk slabs: small at the head
  and tail for pipeline ramp, 8-chunk ~0.85 MB mid-stream for rate).
  Converts run in 2-chunk quanta so a PE chunk waits only on the
  quantum that covers it, and 4 slab buffers keep the stream deep.
- Device, per core: per 128-wide contraction chunk, load x^T as PE
  stationary (32 standalone [128,128] tiles - full-tile weight APs
  measured ~6 us faster than slices of one big tile) and stream the
  bf16 weight rows in bank-aligned segments; accumulate y[tokens,rows]
  in 3 PSUM banks (512/512/352). Bias is replicated across partitions
  on host ([128,RPC] bf16 - partition-matched to the PSUM layout) and
  fused into the drain: each bank drains as one DVE tensor_add (PSUM +
  bias -> bf16 out), so the PE stream carries no bias matmuls. Each
  bank's output DMA starts the moment its drain lands.
- Host: concatenate the 8 row-shards, upcast, reshape.
"""

from contextlib import ExitStack

import numpy as np

ROWS, COLS = 11008, 4096
SCALE = 0.01
T = 128                      # tokens = 4*32
NCORES = 8
RPC = ROWS // NCORES         # 1376 rows per core
CCHUNK = 128                 # contraction tile (partition dim)
NCHUNKS = COLS // CCHUNK     # 32
D_DVE = 824                  # rows widened by DVE; rest by ACT
SLAB_SCHED = [2, 2, 4, 8, 8, 4, 2, 2]
RBLOCKS = [(0, 512), (512, 512), (1024, 352)]
XPIECES = 32                 # x stationary tiles (full-tile = fast LDW)
MAX_PIECE = 512              # cap on PE stream segment width

_cached = {}


def _pieces(d, max_piece=MAX_PIECE):
    """Bank-aligned PE stream segments: (src, lo, hi, bank, boff) where
    src in {d,a}, [lo,hi) is the tile-local column range. Rows [0,d)
    are the DVE group, [d,RPC) the ACT group. Segments longer than
    max_piece are subdivided."""
    segs = [("d", 0, d), ("a", d, RPC)]
    base = {"d": 0, "a": d}
    out = []
    for kind, g0, g1 in segs:
        if g1 <= g0:
            continue
        for b, (r0, rn) in enumerate(RBLOCKS):
            lo, hi = max(g0, r0), min(g1, r0 + rn)
            if lo >= hi:
                continue
            npc = -(-(hi - lo) // max_piece)
            step = -(-(hi - lo) // npc)
            q = lo
            while q < hi:
                qe = min(q + step, hi)
                out.append((kind, q - base[kind], qe - base[kind], b,
                            q - r0))
                q = qe
    first_for_bank, last_for_bank = {}, {}
    for i, p in enumerate(out):
        first_for_bank.setdefault(p[3], i)
        last_for_bank[p[3]] = i
    return out, first_for_bank, last_for_bank


def _build_program(reps=1, loop_reps=0, mode="full", d_dve=D_DVE,
                   sched=None, wbufs=4, xpieces=XPIECES,
                   max_piece=MAX_PIECE, out_bf16=True, conv_q=2,
                   both_sync=True,
                   drain_eng=("scalar", "scalar", "vector")):
    """Build the device program. loop_reps>0 wraps the body in a
    device-side For_i loop (for slope timing). mode: "full" |
    "conv" (DMA+converts) | "dma_both" (DMAs only) | "pe_only"
    (weights loaded once; PE streams slab 0 repeatedly) |
    "dve_only"/"act_only" (convert-rate probes)."""
    import concourse.mybir as mybir
    import concourse.tile as tile
    from concourse import bacc

    WDT = mybir.dt.bfloat16
    ODT = mybir.dt.bfloat16 if out_bf16 else mybir.dt.float32

    sched = list(SLAB_SCHED if sched is None else sched)
    assert sum(sched) == NCHUNKS
    max_slab = max(sched)
    dd = d_dve
    aa = RPC - dd
    pieces, first_for_bank, last_for_bank = _pieces(dd, max_piece)

    nc = bacc.Bacc("TRN2", target_bir_lowering=False, debug=False,
                   enable_asserts=False, num_devices=NCORES)

    # weight row-group shards, host-swizzled to the SBUF slab layout:
    # w*[p, k*R + r] = w^T[k*128 + p, group_base + r] -> every DMA is an
    # identity copy with fully contiguous per-partition DRAM runs.
    wd8 = nc.dram_tensor("wd8", [CCHUNK, NCHUNKS * dd], mybir.dt.int8,
                         kind="ExternalInput").ap()
    wa8 = nc.dram_tensor("wa8", [CCHUNK, NCHUNKS * aa], mybir.dt.int8,
                         kind="ExternalInput").ap()
    x16 = nc.dram_tensor("x16", [CCHUNK, NCHUNKS * T], WDT,
                         kind="ExternalInput").ap()
    # bias replicated across partitions on host: the [128, RPC] layout
    # DMAs at full rate (a [1, RPC] load is a 1-partition crawl) and
    # enters PSUM as one K=128 matmul per bank against a 1/128-valued
    # ones stationary: sum_p bias[r]/128 = bias[r], exactly.
    b16 = nc.dram_tensor("b16", [CCHUNK, RPC], WDT,
                         kind="ExternalInput").ap()
    out = nc.dram_tensor("out", [T, RPC], ODT,
                         kind="ExternalOutput").ap()

    XG = NCHUNKS // xpieces   # chunks per x-stationary tile

    with tile.TileContext(nc) as tc, ExitStack() as ctx:
        const = ctx.enter_context(tc.tile_pool(name="const", bufs=1))
        wdrp = ctx.enter_context(tc.tile_pool(name="wdr", bufs=wbufs))
        warp = ctx.enter_context(tc.tile_pool(name="war", bufs=wbufs))
        wdp = ctx.enter_context(tc.tile_pool(name="wd", bufs=wbufs))
        wap = ctx.enter_context(tc.tile_pool(name="wa", bufs=wbufs))
        psum = ctx.enter_context(tc.tile_pool(name="psum", bufs=3,
                                              space="PSUM"))
        opool = ctx.enter_context(tc.tile_pool(name="o", bufs=2))

        # One-time loads ride the scalar (ACT HWDGE) queue in the
        # preamble; standalone [128,128] x tiles keep the PE weight
        # loads on the fast full-tile path.
        x_tiles = []
        for g in range(xpieces):
            xt = const.tile([CCHUNK, (NCHUNKS // xpieces) * T], WDT,
                            tag=f"x{g}")
            x_tiles.append(xt)
        nc.scalar.dma_start(out=x_tiles[0][:], in_=x16[:, 0:XG * T])
        b_sb = const.tile([CCHUNK, RPC], WDT, tag="b")
        nc.scalar.dma_start(out=b_sb[:], in_=b16[:])
        for g in range(1, xpieces):
            nc.scalar.dma_start(out=x_tiles[g][:],
                                in_=x16[:, g * XG * T:(g + 1) * XG * T])

        def body():
            ps = []
            o_sb = None
            if mode in ("full", "pe_only"):
                for r0, rn in RBLOCKS:
                    pt = psum.tile([T, rn], mybir.dt.float32, tag="acc")
                    ps.append(pt)
                o_sb = opool.tile([T, RPC], ODT, tag="osb")

            slab0 = None
            jmod = sched[0]
            c0 = 0
            for s, sc in enumerate(sched):
                if mode == "pe_only" and s > 0:
                    src = slab0
                elif mode in ("dve_only", "act_only") and s > 0:
                    sc0 = min(sc, sched[0])
                    if mode == "dve_only":
                        wd_sb = wdp.tile([CCHUNK, max_slab, dd], WDT,
                                         tag="wd")
                        nc.vector.tensor_copy(
                            out=wd_sb[:, :sc0, :],
                            in_=slab0["dr"][:, :sc0, :])
                    else:
                        wa_sb = wap.tile([CCHUNK, max_slab, aa], WDT,
                                         tag="wa")
                        nc.scalar.copy(out=wa_sb[:, :sc0, :],
                                       in_=slab0["ar"][:, :sc0, :])
                else:
                    wd_raw = wdrp.tile([CCHUNK, max_slab, dd],
                                       mybir.dt.int8, tag="wdr")
                    nc.sync.dma_start(out=wd_raw[:, :sc, :],
                                      in_=wd8[:, c0 * dd:(c0 + sc) * dd])
                    wa_raw = warp.tile([CCHUNK, max_slab, aa],
                                       mybir.dt.int8, tag="war")
                    aq = nc.sync if both_sync else nc.scalar
                    aq.dma_start(
                        out=wa_raw[:, :sc, :],
                        in_=wa8[:, c0 * aa:(c0 + sc) * aa])
                    wd_sb = wdp.tile([CCHUNK, max_slab, dd], WDT,
                                     tag="wd")
                    wa_sb = wap.tile([CCHUNK, max_slab, aa], WDT,
                                     tag="wa")
                    if mode in ("conv", "full", "pe_only"):
                        # sub-slab converts: PE chunks wait only on the
                        # 2-chunk convert quantum that covers them
                        for q in range(0, sc, conv_q):
                            qe = min(q + conv_q, sc)
                            nc.vector.tensor_copy(
                                out=wd_sb[:, q:qe, :],
                                in_=wd_raw[:, q:qe, :])
                        for q in range(0, sc, conv_q):
                            qe = min(q + conv_q, sc)
                            nc.scalar.copy(out=wa_sb[:, q:qe, :],
                                           in_=wa_raw[:, q:qe, :])
                    src = {"d": wd_sb, "a": wa_sb, "dr": wd_raw,
                           "ar": wa_raw}
                    if s == 0:
                        slab0 = src
                if mode in ("full", "pe_only"):
                    for j in range(sc):
                        k = c0 + j
                        jj = (j % jmod) if (mode == "pe_only" and s > 0) \
                            else j
                        lhsT = x_tiles[k // XG][:, (k % XG) * T:
                                                (k % XG + 1) * T]
                        for i, (kind, lo, hi, b, boff) in enumerate(pieces):
                            stop = (k == NCHUNKS - 1
                                    and last_for_bank[b] == i)
                            # chunk 0's first piece per bank starts the
                            # accumulation group (start zeroes the whole
                            # 2KB bank region)
                            nc.tensor.matmul(
                                ps[b][:, boff:boff + hi - lo], lhsT=lhsT,
                                rhs=src[kind][:, jj, lo:hi],
                                start=(k == 0 and first_for_bank[b] == i),
                                stop=stop)
                            if stop:
                                # drain the bank the moment it stops,
                                # fusing the bias add: the replicated
                                # [128,RPC] bias tile is partition-
                                # matched to the PSUM layout, so the
                                # drain is one DVE tensor_add (PSUM +
                                # bias -> bf16) and the PE never sees
                                # the bias at all. Output DMA follows.
                                r0, rn = RBLOCKS[b]
                                nc.vector.tensor_add(
                                    out=o_sb[:, r0:r0 + rn],
                                    in0=ps[b][:],
                                    in1=b_sb[:, r0:r0 + rn])
                                nc.sync.dma_start(out=out[:, r0:r0 + rn],
                                                  in_=o_sb[:, r0:r0 + rn])
                c0 += sc

        if loop_reps:
            with tc.For_i(0, loop_reps, 1):
                body()
        else:
            for _rep in range(reps):
                body()

    nc.compile()
    return nc


def _get_program():
    if "nc" not in _cached:
        _cached["nc"] = _build_program()
    return _cached["nc"]


def _prep_inputs(x, w_int8, bias, d_dve=D_DVE):
    import ml_dtypes
    BF16 = ml_dtypes.bfloat16
    xs = (x.reshape(T, COLS).astype(np.float32) * np.float32(SCALE))
    # [T, COLS] -> [p, k*T + t] = x^T[k*128+p, t]: the exact SBUF layout
    # the PE stationary slices consume.
    x_dev = np.ascontiguousarray(
        xs.astype(BF16).reshape(T, NCHUNKS, CCHUNK).transpose(2, 1, 0)
    ).reshape(CCHUNK, NCHUNKS * T)

    b2 = bias.astype(BF16).reshape(NCORES, 1, RPC)
    b_sh = np.ascontiguousarray(np.broadcast_to(b2, (NCORES, CCHUNK, RPC)))

    # per-core row groups in SBUF slab layout: [core, p, k*R + r] =
    # w[s*RPC + g0 + r, k*128 + p]
    w8 = w_int8.astype(np.int8).reshape(NCORES, RPC, NCHUNKS, CCHUNK)
    d = d_dve
    wd_sh = np.ascontiguousarray(w8[:, :d].transpose(0, 3, 2, 1)).reshape(
        NCORES, CCHUNK, NCHUNKS * d)
    wa_sh = np.ascontiguousarray(w8[:, d:].transpose(0, 3, 2, 1)).reshape(
        NCORES, CCHUNK, NCHUNKS * (RPC - d))
    return x_dev, b_sh, wd_sh, wa_sh


def kernel(x, w_int8, bias):
    from concourse import bass_utils

    nc = _get_program()
    x_dev, b_sh, wd_sh, wa_sh = _prep_inputs(
        np.asarray(x), np.asarray(w_int8), np.asarray(bias))

    in_maps = [
        {"wd8": wd_sh[s], "wa8": wa_sh[s], "x16": x_dev, "b16": b_sh[s]}
        for s in range(NCORES)
    ]
    res = bass_utils.run_bass_kernel_spmd(nc, in_maps,
                                          core_ids=list(range(NCORES)))
    shards = [res.results[s]["out"] for s in range(NCORES)]
    y = np.concatenate(shards, axis=1).reshape(4, 32, ROWS)
    return np.ascontiguousarray(y.astype(np.float32))


# revision 31
# speedup vs baseline: 1.0411x; 1.0411x over previous
"""CompressedLinear Trainium2 kernel.

Computes y = x @ (w_int8 * 0.01)^T + bias for
  x      [4, 32, 4096]  fp32
  w_int8 [11008, 4096]  int32 (int8 values)
  bias   [11008]        fp32
  y      [4, 32, 11008] fp32

Strategy (tensor-parallel over output rows, 8 NeuronCores):
- The weight payload is int8; stream it from HBM as 1 byte/element
  (5.6 MB/core instead of the baseline's 22.5 MB int32; measured DMA
  rate ~275 GB/s/core) and widen to bf16 on-chip. A single bf16 matmul
  pass suffices: int8 weights are exact in bf16 and x's bf16 rounding
  gives rel err ~1.5e-3 (tolerance is 2e-2); the baseline's hi/lo
  double pass is unnecessary.
- The int8->bf16 widening is split across the two fast elementwise
  engines (measured: DVE 0.74 ns/col, ACT 1.08 ns/col; gpsimd is 5x
  slower and the SDMA cast path costs 2.4x raw streaming, so neither
  is used): rows [0,D) ride the sync HWDGE queue and convert on DVE,
  rows [D,RPC) ride the scalar HWDGE queue and convert on ACT. Row
  groups are separate host-prepared tensors so every DMA is a flat,
  fully contiguous identity copy (2-8 chunk slabs: small at the head
  and tail for pipeline ramp, 8-chunk ~0.85 MB mid-stream for rate).
  Converts run in 1-chunk quanta so a PE chunk waits only on the
  quantum that covers it, and 4 slab buffers keep the stream deep.
- Device, per core: per 128-wide contraction chunk, load x^T as PE
  stationary (32 standalone [128,128] tiles - full-tile weight APs
  measured ~6 us faster than slices of one big tile) and stream the
  bf16 weight rows in bank-aligned segments; accumulate y[tokens,rows]
  in 3 PSUM banks (512/512/352). Bias is replicated across partitions
  on host ([128,RPC] bf16 - partition-matched to the PSUM layout) and
  fused into the drain: each bank drains as one DVE tensor_add (PSUM +
  bias -> bf16 out), so the PE stream carries no bias matmuls. Each
  bank's output DMA starts the moment its drain lands.
- Host: concatenate the 8 row-shards, upcast, reshape.
"""

from contextlib import ExitStack

import numpy as np

ROWS, COLS = 11008, 4096
SCALE = 0.01
T = 128                      # tokens = 4*32
NCORES = 8
RPC = ROWS // NCORES         # 1376 rows per core
CCHUNK = 128                 # contraction tile (partition dim)
NCHUNKS = COLS // CCHUNK     # 32
D_DVE = 824                  # rows widened by DVE; rest by ACT
SLAB_SCHED = [2, 2, 4, 8, 8, 4, 2, 2]
RBLOCKS = [(0, 512), (512, 512), (1024, 352)]
XPIECES = 32                 # x stationary tiles (full-tile = fast LDW)
MAX_PIECE = 512              # cap on PE stream segment width

_cached = {}


def _pieces(d, max_piece=MAX_PIECE):
    """Bank-aligned PE stream segments: (src, lo, hi, bank, boff) where
    src in {d,a}, [lo,hi) is the tile-local column range. Rows [0,d)
    are the DVE group, [d,RPC) the ACT group. Segments longer than
    max_piece are subdivided."""
    segs = [("d", 0, d), ("a", d, RPC)]
    base = {"d": 0, "a": d}
    out = []
    for kind, g0, g1 in segs:
        if g1 <= g0:
            continue
        for b, (r0, rn) in enumerate(RBLOCKS):
            lo, hi = max(g0, r0), min(g1, r0 + rn)
            if lo >= hi:
                continue
            npc = -(-(hi - lo) // max_piece)
            step = -(-(hi - lo) // npc)
            q = lo
            while q < hi:
                qe = min(q + step, hi)
                out.append((kind, q - base[kind], qe - base[kind], b,
                            q - r0))
                q = qe
    first_for_bank, last_for_bank = {}, {}
    for i, p in enumerate(out):
        first_for_bank.setdefault(p[3], i)
        last_for_bank[p[3]] = i
    return out, first_for_bank, last_for_bank


def _build_program(reps=1, loop_reps=0, mode="full", d_dve=D_DVE,
                   sched=None, wbufs=4, xpieces=XPIECES,
                   max_piece=MAX_PIECE, out_bf16=True, conv_q=1,
                   both_sync=True,
                   drain_eng=("scalar", "scalar", "vector")):
    """Build the device program. loop_reps>0 wraps the body in a
    device-side For_i loop (for slope timing). mode: "full" |
    "conv" (DMA+converts) | "dma_both" (DMAs only) | "pe_only"
    (weights loaded once; PE streams slab 0 repeatedly) |
    "dve_only"/"act_only" (convert-rate probes)."""
    import concourse.mybir as mybir
    import concourse.tile as tile
    from concourse import bacc

    WDT = mybir.dt.bfloat16
    ODT = mybir.dt.bfloat16 if out_bf16 else mybir.dt.float32

    sched = list(SLAB_SCHED if sched is None else sched)
    assert sum(sched) == NCHUNKS
    max_slab = max(sched)
    dd = d_dve
    aa = RPC - dd
    pieces, first_for_bank, last_for_bank = _pieces(dd, max_piece)

    nc = bacc.Bacc("TRN2", target_bir_lowering=False, debug=False,
                   enable_asserts=False, num_devices=NCORES)

    # weight row-group shards, host-swizzled to the SBUF slab layout:
    # w*[p, k*R + r] = w^T[k*128 + p, group_base + r] -> every DMA is an
    # identity copy with fully contiguous per-partition DRAM runs.
    wd8 = nc.dram_tensor("wd8", [CCHUNK, NCHUNKS * dd], mybir.dt.int8,
                         kind="ExternalInput").ap()
    wa8 = nc.dram_tensor("wa8", [CCHUNK, NCHUNKS * aa], mybir.dt.int8,
                         kind="ExternalInput").ap()
    x16 = nc.dram_tensor("x16", [CCHUNK, NCHUNKS * T], WDT,
                         kind="ExternalInput").ap()
    # bias replicated across partitions on host: the [128, RPC] layout
    # DMAs at full rate (a [1, RPC] load is a 1-partition crawl) and
    # enters PSUM as one K=128 matmul per bank against a 1/128-valued
    # ones stationary: sum_p bias[r]/128 = bias[r], exactly.
    b16 = nc.dram_tensor("b16", [CCHUNK, RPC], WDT,
                         kind="ExternalInput").ap()
    out = nc.dram_tensor("out", [T, RPC], ODT,
                         kind="ExternalOutput").ap()

    XG = NCHUNKS // xpieces   # chunks per x-stationary tile

    with tile.TileContext(nc) as tc, ExitStack() as ctx:
        const = ctx.enter_context(tc.tile_pool(name="const", bufs=1))
        wdrp = ctx.enter_context(tc.tile_pool(name="wdr", bufs=wbufs))
        warp = ctx.enter_context(tc.tile_pool(name="war", bufs=wbufs))
        wdp = ctx.enter_context(tc.tile_pool(name="wd", bufs=wbufs))
        wap = ctx.enter_context(tc.tile_pool(name="wa", bufs=wbufs))
        psum = ctx.enter_context(tc.tile_pool(name="psum", bufs=3,
                                              space="PSUM"))
        opool = ctx.enter_context(tc.tile_pool(name="o", bufs=2))

        # One-time loads ride the scalar (ACT HWDGE) queue in the
        # preamble; standalone [128,128] x tiles keep the PE weight
        # loads on the fast full-tile path.
        x_tiles = []
        for g in range(xpieces):
            xt = const.tile([CCHUNK, (NCHUNKS // xpieces) * T], WDT,
                            tag=f"x{g}")
            x_tiles.append(xt)
        nc.scalar.dma_start(out=x_tiles[0][:], in_=x16[:, 0:XG * T])
        b_sb = const.tile([CCHUNK, RPC], WDT, tag="b")
        nc.scalar.dma_start(out=b_sb[:], in_=b16[:])
        for g in range(1, xpieces):
            nc.scalar.dma_start(out=x_tiles[g][:],
                                in_=x16[:, g * XG * T:(g + 1) * XG * T])

        def body():
            ps = []
            o_sb = None
            if mode in ("full", "pe_only"):
                for r0, rn in RBLOCKS:
                    pt = psum.tile([T, rn], mybir.dt.float32, tag="acc")
                    ps.append(pt)
                o_sb = opool.tile([T, RPC], ODT, tag="osb")

            slab0 = None
            jmod = sched[0]
            c0 = 0
            for s, sc in enumerate(sched):
                if mode == "pe_only" and s > 0:
                    src = slab0
                elif mode in ("dve_only", "act_only") and s > 0:
                    sc0 = min(sc, sched[0])
                    if mode == "dve_only":
                        wd_sb = wdp.tile([CCHUNK, max_slab, dd], WDT,
                                         tag="wd")
                        nc.vector.tensor_copy(
                            out=wd_sb[:, :sc0, :],
                            in_=slab0["dr"][:, :sc0, :])
                    else:
                        wa_sb = wap.tile([CCHUNK, max_slab, aa], WDT,
                                         tag="wa")
                        nc.scalar.copy(out=wa_sb[:, :sc0, :],
                                       in_=slab0["ar"][:, :sc0, :])
                else:
                    wd_raw = wdrp.tile([CCHUNK, max_slab, dd],
                                       mybir.dt.int8, tag="wdr")
                    nc.sync.dma_start(out=wd_raw[:, :sc, :],
                                      in_=wd8[:, c0 * dd:(c0 + sc) * dd])
                    wa_raw = warp.tile([CCHUNK, max_slab, aa],
                                       mybir.dt.int8, tag="war")
                    aq = nc.sync if both_sync else nc.scalar
                    aq.dma_start(
                        out=wa_raw[:, :sc, :],
                        in_=wa8[:, c0 * aa:(c0 + sc) * aa])
                    wd_sb = wdp.tile([CCHUNK, max_slab, dd], WDT,
                                     tag="wd")
                    wa_sb = wap.tile([CCHUNK, max_slab, aa], WDT,
                                     tag="wa")
                    if mode in ("conv", "full", "pe_only"):
                        # sub-slab converts: PE chunks wait only on the
                        # 2-chunk convert quantum that covers them
                        for q in range(0, sc, conv_q):
                            qe = min(q + conv_q, sc)
                            nc.vector.tensor_copy(
                                out=wd_sb[:, q:qe, :],
                                in_=wd_raw[:, q:qe, :])
                        for q in range(0, sc, conv_q):
                            qe = min(q + conv_q, sc)
                            nc.scalar.copy(out=wa_sb[:, q:qe, :],
                                           in_=wa_raw[:, q:qe, :])
                    src = {"d": wd_sb, "a": wa_sb, "dr": wd_raw,
                           "ar": wa_raw}
                    if s == 0:
                        slab0 = src
                if mode in ("full", "pe_only"):
                    for j in range(sc):
                        k = c0 + j
                        jj = (j % jmod) if (mode == "pe_only" and s > 0) \
                            else j
                        lhsT = x_tiles[k // XG][:, (k % XG) * T:
                                                (k % XG + 1) * T]
                        for i, (kind, lo, hi, b, boff) in enumerate(pieces):
                            stop = (k == NCHUNKS - 1
                                    and last_for_bank[b] == i)
                            # chunk 0's first piece per bank starts the
                            # accumulation group (start zeroes the whole
                            # 2KB bank region)
                            nc.tensor.matmul(
                                ps[b][:, boff:boff + hi - lo], lhsT=lhsT,
                                rhs=src[kind][:, jj, lo:hi],
                                start=(k == 0 and first_for_bank[b] == i),
                                stop=stop)
                            if stop:
                                # drain the bank the moment it stops,
                                # fusing the bias add: the replicated
                                # [128,RPC] bias tile is partition-
                                # matched to the PSUM layout, so the
                                # drain is one DVE tensor_add (PSUM +
                                # bias -> bf16) and the PE never sees
                                # the bias at all. Output DMA follows.
                                r0, rn = RBLOCKS[b]
                                nc.vector.tensor_add(
                                    out=o_sb[:, r0:r0 + rn],
                                    in0=ps[b][:],
                                    in1=b_sb[:, r0:r0 + rn])
                                nc.sync.dma_start(out=out[:, r0:r0 + rn],
                                                  in_=o_sb[:, r0:r0 + rn])
                c0 += sc

        if loop_reps:
            with tc.For_i(0, loop_reps, 1):
                body()
        else:
            for _rep in range(reps):
                body()

    nc.compile()
    return nc


def _get_program():
    if "nc" not in _cached:
        _cached["nc"] = _build_program()
    return _cached["nc"]


def _prep_inputs(x, w_int8, bias, d_dve=D_DVE):
    import ml_dtypes
    BF16 = ml_dtypes.bfloat16
    xs = (x.reshape(T, COLS).astype(np.float32) * np.float32(SCALE))
    # [T, COLS] -> [p, k*T + t] = x^T[k*128+p, t]: the exact SBUF layout
    # the PE stationary slices consume.
    x_dev = np.ascontiguousarray(
        xs.astype(BF16).reshape(T, NCHUNKS, CCHUNK).transpose(2, 1, 0)
    ).reshape(CCHUNK, NCHUNKS * T)

    b2 = bias.astype(BF16).reshape(NCORES, 1, RPC)
    b_sh = np.ascontiguousarray(np.broadcast_to(b2, (NCORES, CCHUNK, RPC)))

    # per-core row groups in SBUF slab layout: [core, p, k*R + r] =
    # w[s*RPC + g0 + r, k*128 + p]
    w8 = w_int8.astype(np.int8).reshape(NCORES, RPC, NCHUNKS, CCHUNK)
    d = d_dve
    wd_sh = np.ascontiguousarray(w8[:, :d].transpose(0, 3, 2, 1)).reshape(
        NCORES, CCHUNK, NCHUNKS * d)
    wa_sh = np.ascontiguousarray(w8[:, d:].transpose(0, 3, 2, 1)).reshape(
        NCORES, CCHUNK, NCHUNKS * (RPC - d))
    return x_dev, b_sh, wd_sh, wa_sh


def kernel(x, w_int8, bias):
    from concourse import bass_utils

    nc = _get_program()
    x_dev, b_sh, wd_sh, wa_sh = _prep_inputs(
        np.asarray(x), np.asarray(w_int8), np.asarray(bias))

    in_maps = [
        {"wd8": wd_sh[s], "wa8": wa_sh[s], "x16": x_dev, "b16": b_sh[s]}
        for s in range(NCORES)
    ]
    res = bass_utils.run_bass_kernel_spmd(nc, in_maps,
                                          core_ids=list(range(NCORES)))
    shards = [res.results[s]["out"] for s in range(NCORES)]
    y = np.concatenate(shards, axis=1).reshape(4, 32, ROWS)
    return np.ascontiguousarray(y.astype(np.float32))
